# revision 50
# baseline (speedup 1.0000x reference)
"""Trainium2 Bass kernel for nn_DistributionLoss_6940667150680 (segment_reduce).

Math: with per-class sums S_c = sum_{i: Y_i=c} w_i and counts n_c,
    L2 = sum_i ||w_i - S_{Y_i}/n_{Y_i}||^2 = sum_i ||w_i||^2 - sum_c ||S_c||^2/n_c
so a single streaming pass over w1 suffices.

Sharding strategy (segment-key sharding): the host routes rows by class --
rows are stably sorted by label and each class is padded with zero rows to a
multiple of 128 so that every 128-row tile belongs to exactly one class.  The
padded tile stream is split evenly across the 8 cores.  Each core then only
needs per-TILE column sums (S_c = sum of its tiles' sums, reduced on host),
which turns the segment reduction into a dense streaming reduce.

v3 layout (per 64-tile chunk, CF = 64*128 = 8192 columns, fp8; the last
chunk may be shorter -- t_core is only rounded to a tile PAIR):
  - DMA: one 1 MB HBM->SBUF transfer per chunk (8 KB per-partition rows).
    The first two chunks are split into quarters/halves so compute starts
    as soon as the first 256 KB lands instead of after the full MB.
  - PE: one DoubleRow fp8 matmul per TILE PAIR: lhsT = [128, 2, 32]
    selector masks (member t of pair p%16 selects psum row 2(p%16)+t),
    rhs = [128, 2, 128] pair slab.  Pairs 0-15 accumulate into psum
    partitions 0-31, pairs 16-31 into partitions 32-63 (the 16 masks are
    shared between the groups) -> [64, 128] tile column sums per chunk.
  - PE (gram): the last NG pairs' squares also run on the PE: lhsT = rhs =
    [128, 2, 64] half-pair slab -> [64, 64] Gram accumulated in a dedicated
    psum bank across the WHOLE kernel; host reads only its diagonal
    (= per-feature sums of squares).
  - ACT: Square activation with accum_out on the first 256*NA columns.
  - DVE: fused square+reduce (scalar_tensor_tensor) on the next 256*NV cols.
  - Evac: DVE tensor_copy moves the [64, 128] psum tile sums to a bf16 SBUF
    bounce ring two chunks late (so DVE never waits on PE), and an SP-ring
    DMA ships it to HBM three chunks later still.  The psum WAR for bank
    reuse rides an SP nop ahead of each w DMA (mm[0]'s wait on the w DMA
    sem then implies the copy finished).
  - Host: fp8 cast + class-sorted layout (input prep), per-class reduction
    of tile sums, counts via bincount, final scalar in float64.
"""

import ml_dtypes
import numpy as np
from contextlib import ExitStack

import concourse.bass as bass
import concourse.tile as tile
from concourse import mybir
from concourse.bass_utils import run_bass_kernel_spmd

N_CORES = 8
D = 128           # feature dim
P = 128           # partitions / rows per tile
CHUNK = 96        # tiles per full DMA chunk
PAIRS = CHUNK // 2
NGRP = 3          # 16-pair mask groups per chunk
NMASK = 16        # distinct selector masks (shared by the two 16-pair groups)
RING = 6          # w-ring depth in chunks
NPS = 7           # psum banks (round-robin per chunk)
CP_LAG = 2        # psum->SBUF copy issued this many chunks late (DVE no-stall)
EV_LAG = 12       # SBUF->HBM evac DMA lag (>= nch: all ship in the tail)
EB = 12           # evac SBUF bounce ring depth (even; evacs ship in pairs)
NA = 19           # pairs squared on ACT  (256*NA columns)
NG = 14           # pairs squared on PE via Gram
# remaining PAIRS - NA - NG pairs squared on DVE
NP_DT = ml_dtypes.float8_e4m3
EV = D            # evac width per chunk: 128 tile-sum cols


def chunk_split(pairs_c, first=False):
    """(na, nv, ng) for a chunk with pairs_c pairs."""
    if first and pairs_c == PAIRS:
        # Chunk 0: ACT/DVE sections sized to single DMA quarters so the
        # engines start as soon as the first 256 KB lands; the (idle,
        # DMA-shadowed) PE picks up the rest via gram.
        return 8, 8, pairs_c - 16
    if pairs_c == PAIRS:
        return NA, PAIRS - NA - NG, NG
    na = max(1, round(NA * pairs_c / PAIRS))
    ng = round(NG * pairs_c / PAIRS)
    nv = pairs_c - na - ng
    if nv < 0:
        ng += nv
        nv = 0
    return na, nv, ng


def build_program(T: int):
    """Per-core program processing T tiles (T % 2 == 0)."""
    f32, f16 = mybir.dt.float32, mybir.dt.float16
    bf16 = mybir.dt.bfloat16
    fdt = mybir.dt.float8e4
    assert T % 2 == 0
    nch = -(-T // CHUNK)
    DR = mybir.MatmulPerfMode.DoubleRow
    cpairs = [PAIRS] * (nch - 1) + [(T - (nch - 1) * CHUNK) // 2]
    assert cpairs[-1] >= 1
    splits = [chunk_split(pc, first=(i == 0))
              for i, pc in enumerate(cpairs)]
    last_gram = max((c for c in range(nch) if splits[c][2] > 0), default=None)
    CF = CHUNK * D

    nc = bass.Bass()
    w_in = nc.dram_tensor("w", [P, T * D], fdt, kind="ExternalInput")
    masks_in = nc.dram_tensor("masks", [P, NMASK * 2 * 32], fdt, kind="ExternalInput")
    ts_out = nc.dram_tensor("ts_out", [32, nch * NGRP * EV], bf16, kind="ExternalOutput")
    sqa_out = nc.dram_tensor("sqa_out", [P, max(nch, 1)], f32, kind="ExternalOutput")
    sqv_out = nc.dram_tensor("sqv_out", [P, max(nch, 1)], f32, kind="ExternalOutput")
    gram_out = nc.dram_tensor("gram_out", [64, 64], f32, kind="ExternalOutput")

    def dep(frm, to, why):
        tile.add_dep_helper(
            getattr(frm, "ins", frm), getattr(to, "ins", to), reason=why
        )

    def demote(inst, dep_insts):
        """Move provably-redundant sync deps to nosync (ordering only):
        same-engine WAW/WAR (in-order engines) and deps transitively covered
        by another emitted wait (ISA structs hold one sync wait each)."""
        inst = getattr(inst, "ins", inst)
        drop = set()
        for d in dep_insts:
            if d is None:
                continue
            drop.add(getattr(d, "ins", d).name)
        syncs = inst.take_sync_dependencies()
        nosyncs = inst.take_nosync_dependencies()
        for name in drop & set(syncs):
            syncs.discard(name)
            nosyncs.add(name)
        inst.set_sync_dependencies(syncs)
        inst.set_nosync_dependencies(nosyncs)

    # Pin each engine queue to emission order with demoted (nosync) chain
    # edges: the tile scheduler may otherwise reorder within a queue, which
    # breaks every "covered transitively via in-order engine" argument below.
    last_on = {}

    def chain(inst, engine):
        prev = last_on.get(engine)
        if prev is not None:
            dep(inst, prev, "queue order")
            demote(inst, [prev])
        last_on[engine] = inst
        return inst

    with tile.TileContext(nc) as tc, ExitStack() as ctx:
        const = ctx.enter_context(tc.tile_pool(name="const", bufs=1))
        psum = ctx.enter_context(tc.tile_pool(name="psum", bufs=1, space="PSUM"))

        masks_sb = const.tile([P, NMASK, 2, 32], fdt, name="masks_sb")
        w_ring = const.tile([P, RING, PAIRS, 2, D], fdt, name="w_ring")
        sqa_cols = const.tile([P, max(nch, 1)], f32, name="sqa_cols")
        sqv_cols = const.tile([P, max(nch, 1)], f32, name="sqv_cols")
        scrA = const.tile([P, 2, 256 * NA], f16, name="scrA")
        scrV = const.tile([P, 2, 256 * max(PAIRS - NA - NG, 1)], f16, name="scrV")
        evac_sb = const.tile([32, EB, NGRP * EV], bf16, name="evac_sb")
        gram_sb = const.tile([64, 64], f32, name="gram_sb")

        # [32, NGRP*128]: the 16-pair groups sit side by side in the free
        # dim (DR matmuls must target psum partition 0).
        pst = [psum.tile([32, NGRP * EV], f32, name=f"pst{k}") for k in range(NPS)]
        # One gram bank accumulates squares across the whole kernel; its
        # diagonal is read once at the end.
        gps = psum.tile([64, 64], f32, name="gps")

        # Masks ride the ACT HWDGE ring so they don't delay the first w chunk
        # on the SP ring (ACT is idle until the first chunk lands anyway).
        dma_masks = chain(nc.scalar.dma_start(out=masks_sb, in_=masks_in[:, :]),
                          "act")

        dmas = {}      # chunk -> list of (pair_lo, pair_hi, dma_inst)
        readers = {}   # chunk -> instructions that read its ring slot
        acts = {}
        ttrs = {}
        pe_last = {}   # chunk -> last PE instruction of the chunk
        sel_last = {}
        cps = {}       # chunk -> DVE psum->SBUF copy
        evacs = {}     # chunk -> SBUF->HBM evac DMA

        def sec_dma(c, pair_lo):
            """The w-DMA section of chunk c whose LAST overlapping section
            covers pair_lo..; consumers keep only this section's dep (the SP
            ring FIFO implies all earlier sections of the chunk drained)."""
            for lo, hi, dd in dmas[c]:
                if lo <= pair_lo < hi:
                    return dd
            raise AssertionError((c, pair_lo))

        def all_dmas():
            return [dd for c2 in dmas for (_, _, dd) in dmas[c2]]

        def emit_cp(c):
            """DVE: psum bank of chunk c -> SBUF bounce slot c%EB (bf16).  A
            DVE nop carries the bounce-slot WAR (evac DMA pair covering chunk
            c-EB drained); the copy itself carries the PE-done wait."""
            if c - EB >= 0 and (c - EB) // 2 in evacs:
                nd = chain(nc.vector.engine_nop(), "dve")
                dep(nd, evacs[(c - EB) // 2], "bounce slot free")
            cp = chain(
                nc.vector.tensor_copy(evac_sb[:, c % EB, :], pst[c % NPS][:, :]),
                "dve",
            )
            dep(cp, pe_last[c], "psum ready")
            demote(cp, [r for c2 in readers for r in readers[c2]
                        if r is not pe_last[c]]
                   + list(cps.values()) + list(evacs.values()))
            cps[c] = cp

        def emit_evac(m):
            """SP: bounce slots of chunk pair (2m, 2m+1) -> HBM in one DMA
            (adjacent slots -- EB is even), behind a nop carrying the last
            copy's wait."""
            c0 = 2 * m
            c1 = min(2 * m + 1, nch - 1)
            spn = chain(nc.sync.nop(nofuse=True, hint=f"ev{m}"), "sp")
            dep(spn, cps[c1], "copies done")
            ev = chain(
                nc.sync.dma_start(
                    out=ts_out[:, c0 * NGRP * EV : (c1 + 1) * NGRP * EV],
                    in_=evac_sb[:, c0 % EB : c0 % EB + (c1 - c0 + 1), :],
                ),
                "sp",
            )
            dep(ev, spn, "after producer nop")
            demote(ev, [spn, cps[c0], cps[c1]] + all_dmas()
                   + list(evacs.values()))
            evacs[m] = ev

        for c in range(nch):
            j = c % RING
            pairs_c = cpairs[c]
            na, nv, ng = splits[c]
            # Ring-slot WAR: carry waits on SP nops (the chain edges keep the
            # DMA behind them in the SP queue, so their hardware waits also
            # protect it).
            if c >= RING:
                n1 = chain(nc.sync.nop(nofuse=True, hint=f"war{c}a"), "sp")
                dep(n1, acts[c - RING], "act reader done")
                if (c - RING) in ttrs:
                    n1b = chain(nc.sync.nop(nofuse=True, hint=f"war{c}c"), "sp")
                    dep(n1b, ttrs[c - RING], "ttr reader done")
                n2 = chain(nc.sync.nop(nofuse=True, hint=f"war{c}b"), "sp")
                dep(n2, pe_last[c - RING], "pe reader done")
            # psum-bank WAR (chunk c-NPS's copy) rides a PE nop ahead of mm[0].
            # Keeping it off the SP queue preserves DMA prefetch during the
            # pipeline fill (the copy chain lags behind the data stream, and a
            # gated SP queue would throttle every later w DMA).
            # w DMA, split into sections for the first chunks so compute can
            # start before the whole MB lands.
            nsec = 4 if c == 0 else (2 if c == 1 else 1)
            nsec = min(nsec, pairs_c)
            dmas[c] = []
            bounds = [pairs_c * s // nsec for s in range(nsec + 1)]
            prior_readers = [r for k in readers for r in readers[k]]
            for lo, hi in zip(bounds, bounds[1:]):
                dd = chain(
                    nc.sync.dma_start(
                        out=w_ring[:, j, lo:hi, :, :],
                        in_=w_in[:, c * CF + lo * 2 * D : c * CF + hi * 2 * D],
                    ),
                    "sp",
                )
                # WAR waits live on the nops just above; DMA-vs-DMA WAW is
                # ordered by the HWDGE ring (FIFO per issuing engine).
                demote(dd, all_dmas() + list(evacs.values()) + [dma_masks]
                       + prior_readers)
                dmas[c].append((lo, hi, dd))
            readers[c] = []

            # PE: selector sums.  One DoubleRow matmul per tile pair; member t
            # of pair p lands in psum row 32*(p//16) + 2*(p%16) + t.  Each
            # mm waits (at most) on its section's DMA sem; that also covers
            # the masks DMA (first LDW carries that wait separately) and the
            # psum evac of chunk c-NPS via the SP ring FIFO.
            pt = pst[c % NPS]
            pe_covered = [dma_masks, pe_last.get(c - 1), pe_last.get(c - NPS),
                          sel_last.get(c - NPS), cps.get(c - NPS)]
            kept = set()

            def emit_gram(gi, h, first=False):
                p = pairs_c - ng + gi
                gm = chain(
                    nc.tensor.matmul(
                        gps[:, :],
                        lhsT=w_ring[:, j, p, :, h * 64 : (h + 1) * 64],
                        rhs=w_ring[:, j, p, :, h * 64 : (h + 1) * 64],
                        start=(c == 0 and gi == 0 and h == 0),
                        stop=(c == last_gram and gi == ng - 1 and h == 1),
                        perf_mode=DR,
                        skip_group_check=True,
                    ),
                    "pe",
                )
                sd = sec_dma(c, p)
                if first or sd.ins.name not in kept:
                    kept.add(sd.ins.name)
                    demote(gm, [d for d in all_dmas() if d is not sd]
                           + pe_covered + list(sel_last.values()))
                else:
                    demote(gm, all_dmas() + pe_covered
                           + list(sel_last.values()))
                readers[c].append(gm)

            # psum-bank WAR (chunk c-NPS's copy must finish before mm[0]
            # re-zeroes the bank): hoist one gram matmul (it only writes the
            # gram bank) to carry this chunk's DMA wait, so the first
            # selector matmul's single sync wait can be the copy instead.
            # Keeping this off the SP queue preserves DMA prefetch during the
            # pipeline fill (the copy chain lags behind the data stream).
            hoist = (c - NPS) in cps and ng > 0 and len(dmas[c]) == 1
            if hoist:
                emit_gram(0, 0, first=True)
            elif (c - NPS) in cps:
                n3 = chain(nc.sync.nop(nofuse=True, hint=f"war{c}p"), "sp")
                dep(n3, cps[c - NPS], "psum bank free")

            for p in range(pairs_c):
                g, q = divmod(p, NMASK)
                mm = chain(
                    nc.tensor.matmul(
                        pt[:, g * D : (g + 1) * D],
                        lhsT=masks_sb[:, q, :, :],
                        rhs=w_ring[:, j, p, :, :],
                        start=(p == 0),
                        stop=(p == pairs_c - 1),
                        perf_mode=DR,
                        skip_group_check=True,
                    ),
                    "pe",
                )
                sd = sec_dma(c, p)
                if p == 0 and hoist:
                    # single wait: the copy of chunk c-NPS (bank free); the
                    # data wait rides the hoisted gram mm (PE is in-order).
                    kept.add(sd.ins.name)
                    demote(mm, [d for d in all_dmas()]
                           + [x for x in pe_covered
                              if x is not cps.get(c - NPS)])
                elif sd.ins.name not in kept:
                    kept.add(sd.ins.name)
                    demote(mm, [d for d in all_dmas() if d is not sd]
                           + pe_covered)
                else:
                    demote(mm, all_dmas() + pe_covered)
                readers[c].append(mm)
            sel_last[c] = mm
            # PE: gram squares for the last ng pairs (two half-feature
            # matmuls per pair).  One accumulation group spans the whole
            # kernel in its own psum bank; the diagonal is read once at the
            # end.
            for gi in range(ng):
                for h in range(2):
                    if hoist and gi == 0 and h == 0:
                        continue
                    emit_gram(gi, h)
            pe_last[c] = readers[c][-1]

            # ACT: squares of the first 256*na columns, accumulated per
            # chunk.  Keep only the dep on the LAST section it reads (ring
            # FIFO covers the earlier ones).
            act = chain(
                nc.scalar.activation(
                    scrA[:, c % 2, 0 : 256 * na],
                    w_ring[:, j, 0:na, :, :],
                    mybir.ActivationFunctionType.Square,
                    accum_out=sqa_cols[:, c : c + 1],
                ),
                "act",
            )
            sd = sec_dma(c, na - 1)
            demote(act, list(acts.values()) + [dma_masks]
                   + [d for d in all_dmas() if d is not sd])
            acts[c] = act
            readers[c].append(act)

            # Lagged psum->SBUF copy, emitted BEFORE this chunk's ttr so the
            # SP-side waits on it (psum WAR, evac) never chain behind a fresh
            # ttr in the in-order DVE queue.
            if c >= CP_LAG:
                emit_cp(c - CP_LAG)

            # DVE: fused square+reduce of the middle 256*nv columns.
            if nv > 0:
                ttr = chain(
                    nc.vector.scalar_tensor_tensor(
                        out=scrV[:, c % 2, 0 : 256 * nv],
                        in0=w_ring[:, j, na : na + nv, :, :],
                        scalar=1.0,
                        in1=w_ring[:, j, na : na + nv, :, :],
                        op0=mybir.AluOpType.mult,
                        op1=mybir.AluOpType.mult,
                        accum_out=sqv_cols[:, c : c + 1],
                    ),
                    "dve",
                )
                sd = sec_dma(c, na + nv - 1)
                demote(ttr, list(ttrs.values())
                       + [d for d in all_dmas() if d is not sd])
                ttrs[c] = ttr
                readers[c].append(ttr)

            # Lagged SBUF->HBM evac: by now the copies of pair c-EV_LAG are
            # long done, so the SP queue never stalls and the w-DMA prefetch
            # stays deep even during the pipeline fill.
            if c >= EV_LAG and (c - EV_LAG) % 2 == 1:
                emit_evac((c - EV_LAG) // 2)

        # Tail flush: finish the copies (cadence order), then ship any
        # remaining evac pairs.  Slot-reuse ordering needs no care here --
        # nothing writes the bounce ring after the last copy.
        for cc in range(nch, nch + CP_LAG):
            if 0 <= cc - CP_LAG < nch and (cc - CP_LAG) not in cps:
                emit_cp(cc - CP_LAG)
        for m in range(-(-nch // 2)):
            if m not in evacs:
                emit_evac(m)

        # Final gram readout: DVE copy psum -> SBUF once, then DMA out.
        gcp = chain(nc.vector.tensor_copy(gram_sb[:, :], gps[:, :]), "dve")
        glast = pe_last[last_gram] if last_gram is not None else pe_last[nch - 1]
        dep(gcp, glast, "gram done")
        demote(gcp, [r for c2 in readers for r in readers[c2] if r is not glast]
               + list(cps.values()) + list(evacs.values()))

        # Outputs: each DMA waits on the last producer via an SP nop.
        outs = []
        for name, buf, src, last in (
            ("sqa", sqa_out, sqa_cols, acts[nch - 1]),
            ("sqv", sqv_out, sqv_cols, ttrs.get(max(ttrs) if ttrs else 0)),
            ("gram", gram_out, gram_sb, gcp),
        ):
            if last is None:
                continue
            spn = chain(nc.sync.nop(nofuse=True, hint=f"out_{name}"), "sp")
            dep(spn, last, f"{name} ready")
            od = chain(nc.sync.dma_start(out=buf[:, :], in_=src), "sp")
            dep(od, spn, "after producer nop")
            demote(od, [spn, last] + all_dmas() + outs
                   + list(evacs.values()) + list(acts.values())
                   + list(ttrs.values()) + list(cps.values()))
            outs.append(od)

        # Tail sync: cover every proc with single-wait SP nops.
        tails = [pe_last[nch - 1], acts[nch - 1], gcp] + outs + [evacs[max(evacs)]]
        if ttrs:
            tails.append(ttrs[max(ttrs)])
        for t in tails:
            nop = chain(nc.sync.nop(nofuse=True, hint="tailcover"), "sp")
            dep(nop, t, "tail")

    # The kernel-tail drain waits on every proc; its NOP struct cannot hold
    # that many sync waits and the SP-queue nops above already cover them.
    for blk in nc.m.functions[0].blocks:
        for inst in blk.instructions:
            if not isinstance(inst, mybir.InstDrain):
                continue
            si = inst.sync_info
            if si is None or len(si.on_wait) <= 2:
                continue
            inst.sync_info = mybir.SyncInfo(on_wait=[], on_update=list(si.on_update))

    return nc


def prepare_inputs(w1: np.ndarray, Y: np.ndarray, num_classes: int):
    """Class-sorted, per-class tile-padded, per-core partition-major fp8."""
    n = w1.shape[0]
    counts = np.bincount(Y, minlength=num_classes).astype(np.int64)
    tpc_class = (counts + P - 1) // P          # tiles per class
    pad_start = np.zeros(num_classes + 1, dtype=np.int64)
    np.cumsum(tpc_class, out=pad_start[1:])
    tt = int(pad_start[-1])                    # total real tiles
    t_core = -(-tt // N_CORES)                 # ceil
    t_core = -(-t_core // 2) * 2               # round up to a tile pair
    t_total = t_core * N_CORES

    order = np.argsort(Y, kind="stable")
    y_sorted = Y[order]
    class_start = np.zeros(num_classes, dtype=np.int64)
    class_start[1:] = np.cumsum(counts)[:-1]
    rank = np.arange(n, dtype=np.int64) - class_start[y_sorted]
    dest = pad_start[y_sorted] * P + rank

    w16 = np.zeros((t_total * P, D), dtype=NP_DT)
    w16[dest] = w1[order].astype(NP_DT)

    # selector masks: masks[k, q, t, m] = (m == 2q+t), laid out
    # [P, NMASK*2*32]
    m1 = np.zeros((NMASK, 2, 32), dtype=NP_DT)
    for q in range(NMASK):
        m1[q, 0, 2 * q] = 1
        m1[q, 1, 2 * q + 1] = 1
    masks = np.ascontiguousarray(
        np.broadcast_to(m1.reshape(1, NMASK * 2 * 32), (P, NMASK * 2 * 32))
    )
    in_maps = []
    for k in range(N_CORES):
        blk = w16[k * t_core * P : (k + 1) * t_core * P]
        wk = np.ascontiguousarray(
            blk.reshape(t_core, P, D).transpose(1, 0, 2).reshape(P, t_core * D)
        )
        in_maps.append({"w": wk, "masks": masks})
    return in_maps, t_core, pad_start, counts


def combine(results, t_core, pad_start, counts, n_total):
    """Host-side: tile sums -> class sums -> final scalar, in float64."""
    nch = -(-t_core // CHUNK)
    # ts_out rows: psum row 2q+t of group g in chunk c = tile 64c + 32g + 2q+t;
    # group g occupies cols [g*EV, (g+1)*EV) of chunk block c.
    tile_sums = np.concatenate(
        [
            r["ts_out"].astype(np.float64)
            .reshape(32, nch, NGRP, EV).transpose(1, 2, 0, 3)
            .reshape(nch * CHUNK, EV)[:t_core]
            for r in results
        ],
        axis=0,
    )  # [8 * t_core, D] -- but each core block is t_core rows
    num_classes = len(counts)
    cpairs = [PAIRS] * (nch - 1) + [(t_core - (nch - 1) * CHUNK) // 2]
    splits = [chunk_split(pc) for pc in cpairs]
    va = [c for c in range(nch) if splits[c][0] > 0]
    vv = [c for c in range(nch) if splits[c][1] > 0]
    totsq = 0.0
    for r in results:
        totsq += float(r["sqa_out"].astype(np.float64)[:, va].sum())
        totsq += float(r["sqv_out"].astype(np.float64)[:, vv].sum())
        totsq += float(np.trace(r["gram_out"].astype(np.float64)))
    # per-class sums: classes are tile-aligned runs of tile_sums
    seg = np.add.reduceat(tile_sums[: pad_start[-1]], pad_start[:-1], axis=0) \
        if pad_start[-1] > 0 else np.zeros((num_classes, EV))
    # reduceat quirk: empty segments (pad_start[c]==pad_start[c+1]) copy the
    # row at that index instead of 0 -- mask them out via counts.
    nz = counts > 0
    s = seg[nz]
    corr = float(((s * s).sum(axis=1) / counts[nz]).sum())
    return np.float32((totsq - corr) / n_total)


def run_sharded(w1: np.ndarray, Y: np.ndarray, num_classes: int, trace: bool = False):
    w1 = np.ascontiguousarray(np.asarray(w1, dtype=np.float32))
    Y = np.asarray(Y).astype(np.int64)
    in_maps, t_core, pad_start, counts = prepare_inputs(w1, Y, num_classes)
    nc = build_program(t_core)
    out = run_bass_kernel_spmd(nc, in_maps, list(range(N_CORES)), trace=trace)
    value = combine(out.results, t_core, pad_start, counts, w1.shape[0])
    return value, out


def kernel(w1, Y, num_classes=None):
    w1 = np.asarray(w1, dtype=np.float32)
    Y = np.asarray(Y)
    c = int(np.asarray(num_classes)) if num_classes is not None else 1000
    assert w1.ndim == 2 and w1.shape[1] == D
    value, _ = run_sharded(w1, Y, c, trace=False)
    return value
